# revision 13
# baseline (speedup 1.0000x reference)
"""Trainium2 Bass kernel for nn_Attention_62362925138174 (v4).

Reference (per batch b, xf = x[b].reshape(C, N), N = H*W = 4096):
    q = Wq @ xf; k = Wk @ xf; v = Wv @ xf
    score[n, m] = q[:, n] . k[:, m]
    P = softmax(score, axis=n)             (per-column softmax)
    att = gamma * (v @ P) + xf

Kernel strategy (8 cores = 4 batches x 2 column-halves of N):
  - score = xf^T (Wq^T Wk) xk via kg = G @ xk, bf16 (full PE column rate).
  - E = exp(score) fp8(e4m3): ScalarE real Exp + VectorE Schraudolph
    bit-trick, balanced by the measured cost models
    Act (FD+352)/1.2ns, DVE (FD+120)/0.96ns; instructions span TWO
    adjacent PSUM ring slots when the 3-slot ring allows (2/3 of pairs)
    to amortize fixed overhead.
  - PV runs fp8 DoubleRow with vaug = [gamma*v^T | ones64]: output rows
    0:64 accumulate gamma*(v E), rows 64:128 accumulate colsum(E)
    broadcast across 64 partitions FOR FREE (PE output rows cost
    nothing) -- this kills the gpsimd partition_broadcast + its ~6us
    IRAM library loads that serialized v3.
  - Tail per chunk: reciprocal_approx_fast on the den rows, then
    mul + residual-add on DVE, emitted early in the next chunk so the
    O PSUM buffer is released before the next chunk's first PV needs
    it. No gpsimd ops anywhere in the kernel.
  - Inputs: 2 big DMAs (sync + scalar HWDGE rings); xkf input dropped,
    the residual reads the bf16 xfp (error budget ~1e-3 << 2e-2).
"""

import numpy as np

import concourse.bass as bass
import concourse.bacc as bacc
import concourse.tile as tile
from concourse import mybir
from concourse.bass_utils import run_bass_kernel_spmd

B, C, H, W = 4, 64, 64, 64
N = H * W            # 4096
MHALF = N // 2       # 2048 columns of score/output per core
NT = N // 128        # 32 row-tiles of the score matrix
NP = NT // 2         # 16 row-tile pairs
N_CORES = 8

F32 = mybir.dt.float32
BF16 = mybir.dt.bfloat16
FP8 = mybir.dt.float8e4
I8 = mybir.dt.int8
NP_FP8 = mybir.dt.np(FP8)
NP_BF16 = mybir.dt.np(BF16)

EXP = mybir.ActivationFunctionType.Exp
MULT = mybir.AluOpType.mult
ADD = mybir.AluOpType.add
DR = mybir.MatmulPerfMode.DoubleRow

# Schraudolph constants for e4m3 bit-pattern exp (DVE f32->int8 convert
# rounds to nearest).
SCH_A = 8.0 / float(np.log(2.0))       # 11.5416
SCH_B = 56.0 - 0.349                   # RMS-optimal Schraudolph shift

_PROGRAM = None

# exp column split per tile: Act cost (X+352)/1.2 ns, DVE cost
# (1024-X+120)/0.96 ns; X=520 balances at ~727/650 ns with the DVE also
# carrying the per-chunk tail (rcp+mul).
X_ACT = 520
PV_LAG = 5            # pairs of lag between exp and PV emission


def _build_program() -> bass.Bass:
    nc = bacc.Bacc()

    xfp_d = nc.declare_dram_parameter("xfp", [128, N], BF16, isOutput=False)
    # gt16 (cols 0:128) and wv16 (cols 128:192) combined: one DMA
    wg_d = nc.declare_dram_parameter("wg", [128, 192], BF16, isOutput=False)
    out_d = nc.declare_dram_parameter("out", [C, MHALF], F32, isOutput=True)

    from concourse.hw_specs import get_activation_tables

    act_sets = list(get_activation_tables(nc.m.arch))
    nle_id = act_sets.index("natural_log_exp_and_others")

    from contextlib import ExitStack

    with ExitStack() as stack:
        tc = stack.enter_context(tile.TileContext(nc))
        sing = stack.enter_context(tc.tile_pool(name="sing", bufs=1))
        apool = stack.enter_context(tc.tile_pool(name="apool", bufs=2))
        psS = stack.enter_context(tc.tile_pool(name="psS", bufs=1, space="PSUM"))
        psO = stack.enter_context(tc.tile_pool(name="psO", bufs=1, space="PSUM"))

        nc.scalar.add_instruction(
            mybir.InstLoadActFuncSet(
                name=nc.get_next_instruction_name(),
                act_func_set_id=nle_id,
                ins=[],
                outs=[],
            )
        )

        # ---- input DMAs: each HWDGE ring pays ~2us fixed per dma_start
        # (FIFO), so: sync ring = one big xfp[0:2048]; scalar ring =
        # weights then xfp[2048:4096]. Act's queue is idle this early. ----
        wg_sb = sing.tile([128, 192], BF16, name="wg_sb")
        nc.scalar.dma_start(out=wg_sb, in_=wg_d[:, :])
        xfp_sb = sing.tile([128, N], BF16, name="xfp_sb")
        nc.sync.dma_start(out=xfp_sb[:, 0:2048], in_=xfp_d[:, 0:2048])
        nc.scalar.dma_start(out=xfp_sb[:, 2048:4096], in_=xfp_d[:, 2048:4096])
        gt16_sb = wg_sb[:, 0:128]
        wv16_sb = wg_sb[:, 128:192]

        # ---- persistent SBUF tiles ----
        kg_sb = sing.tile([128, MHALF], BF16, name="kg_sb")
        # vaug[n, 0:64] = (gamma Wv xf)^T, vaug[n, 64:128] = 1.0 so PV's
        # DoubleRow output rows 64:128 all accumulate colsum(E).
        vaug_sb = sing.tile([128, NT, 128], FP8, name="vaug_sb")
        nc.vector.memset(vaug_sb[:, :, 64:128], 1.0)
        E_sb = sing.tile([128, NT, 1024], FP8, name="E_sb")

        # gpsimd tensor-lib preload: the first gpsimd tensor op pays a
        # ~6us IRAM library load; trigger it now so the ch0-tail add is
        # cheap. Keep gpsimd on this ONE library for the whole kernel.
        gdum = sing.tile([1, 16], F32, name="gdum")
        nc.vector.memset(gdum, 1.0)
        nc.gpsimd.tensor_add(gdum, gdum, gdum)

        # S ring: one PSUM tile, 3 slots of [128, 1024] (banks 0-5);
        # slice-level deps give ring semantics. Slot pairs (0,1) and
        # (1,2) are contiguous -> single wide exp instructions.
        S = psS.tile([128, 3, 1024], F32, name="S_ring")
        O_ps = psO.tile([128, 1024], F32, name="O_ps")

        # ---- kg = G @ xk [128, MHALF] bf16 (rows 64+ zero via padded
        # gt16). Uses S ring slots as staging before the main loop. ----
        for h in range(2):
            kslot = S[:, h, :]
            for cc in range(2):
                lo = h * 1024 + cc * 512
                nc.tensor.matmul(
                    kslot[:, cc * 512 : (cc + 1) * 512],
                    lhsT=gt16_sb,
                    rhs=xfp_sb[:, lo : lo + 512],
                    start=True,
                    stop=True,
                )
            lo = h * 1024
            nc.scalar.copy(out=kg_sb[:, lo : lo + 512], in_=kslot[:, 0:512])
            nc.vector.tensor_copy(
                out=kg_sb[:, lo + 512 : lo + 1024], in_=kslot[:, 512:1024]
            )

        # ---- vaug v-part: vt = xfp_tile^T @ wv16 in batches of 16 tiles
        # through S slot 2 then slot 0 (slot 0's kg read is done by the
        # time the second batch's matmuls land). Also serves as the PE
        # HAM warm-up burst. ----
        for vv in range(2):
            vslot = S[:, 2 - 2 * vv, :]
            for i in range(16):
                t = vv * 16 + i
                nc.tensor.matmul(
                    vslot[:, i * 64 : (i + 1) * 64],
                    lhsT=xfp_sb[:, t * 128 : (t + 1) * 128],
                    rhs=wv16_sb,
                    start=True,
                    stop=True,
                )
            vtv = vslot.rearrange("p (i u) -> p i u", u=64)
            nc.scalar.copy(
                out=vaug_sb[:, vv * 16 : vv * 16 + 8, 0:64], in_=vtv[:, 0:8, :]
            )
            nc.vector.tensor_copy(
                out=vaug_sb[:, vv * 16 + 8 : vv * 16 + 16, 0:64], in_=vtv[:, 8:16, :]
            )

        def tail_steps(ch):
            """Per-chunk tail (overlapped with the next chunk's loop):
            den rows 64:128 of O are the broadcast colsum. Act stages
            den, DVE does rcp + mul, GpSimd (tensor-lib only, preloaded)
            does the residual add, sync ring DMAs out. O is released
            after steps 1+3."""
            den_sb = apool.tile([C, 1024], F32, tag="den", name="den_sb")
            rcpb = apool.tile([C, 1024], F32, tag="rcpb", name="rcpb")
            tmp = apool.tile([C, 1024], F32, tag="tmp", name="tmp")
            att = apool.tile([C, 1024], F32, tag="att", name="att")
            ocols = slice(ch * 1024, (ch + 1) * 1024)
            yield lambda: nc.scalar.copy(out=den_sb, in_=O_ps[64:128, :])
            yield lambda: (
                nc.vector.reciprocal_approx_fast(out=rcpb[:, 0:512], in_=den_sb[:, 0:512]),
                nc.vector.reciprocal_approx_fast(out=rcpb[:, 512:1024], in_=den_sb[:, 512:1024]),
            )
            yield lambda: nc.vector.tensor_mul(tmp, O_ps[0:C, :], rcpb)
            yield lambda: nc.gpsimd.tensor_add(att, tmp, xfp_sb[0:C, ocols])
            yield lambda: nc.sync.dma_start(out=out_d[:, ocols], in_=att)

        def final_tail(ch):
            """End-of-kernel tail: pipelined in column halves, DVE-only
            chain (latency matters, nothing left to overlap)."""
            den_sb = apool.tile([C, 1024], F32, tag="den", name="den_sb")
            rcpb = apool.tile([C, 1024], F32, tag="rcpb", name="rcpb")
            tmp = apool.tile([C, 1024], F32, tag="tmp", name="tmp")
            att = apool.tile([C, 1024], F32, tag="att", name="att")
            OUTQ = [nc.sync, nc.scalar]
            for cc in range(2):
                sl = slice(cc * 512, (cc + 1) * 512)
                osl = slice(ch * 1024 + cc * 512, ch * 1024 + (cc + 1) * 512)
                nc.scalar.copy(out=den_sb[:, sl], in_=O_ps[64:128, sl])
                nc.vector.reciprocal_approx_fast(out=rcpb[:, sl], in_=den_sb[:, sl])
                nc.vector.tensor_mul(tmp[:, sl], O_ps[0:C, sl], rcpb[:, sl])
                nc.vector.tensor_add(att[:, sl], tmp[:, sl], xfp_sb[0:C, osl])
                OUTQ[cc].dma_start(out=out_d[:, osl], in_=att[:, sl])

        def emit_exp(t):
            slot = S[:, t % 3, :]
            nc.scalar.activation(
                out=E_sb[:, t, 0:X_ACT], in_=slot[:, 0:X_ACT], func=EXP)
            nc.vector.tensor_scalar(
                out=E_sb.bitcast(I8)[:, t, X_ACT:1024],
                in0=slot[:, X_ACT:1024],
                scalar1=SCH_A, scalar2=SCH_B, op0=MULT, op1=ADD,
            )

        def emit_pv(j):
            vpair = vaug_sb[:, 2 * j : 2 * j + 2, :]
            for cc in range(2):
                nc.tensor.matmul(
                    O_ps[:, cc * 512 : (cc + 1) * 512],
                    lhsT=vpair,
                    rhs=E_sb[:, 2 * j : 2 * j + 2, cc * 512 : (cc + 1) * 512],
                    start=(j == 0),
                    stop=(j == NP - 1),
                    perf_mode=DR,
                )

        prev_tail = None
        for ch in range(2):
            for p in range(NP):
                for t in (2 * p, 2 * p + 1):
                    slot = S[:, t % 3, :]
                    lhsT_t = xfp_sb[:, t * 128 : (t + 1) * 128]
                    for cc in range(2):
                        nc.tensor.matmul(
                            slot[:, cc * 512 : (cc + 1) * 512],
                            lhsT=lhsT_t,
                            rhs=kg_sb[:, ch * 1024 + cc * 512 : ch * 1024 + (cc + 1) * 512],
                            start=True,
                            stop=True,
                        )
                    emit_exp(t)
                # one lagged tail step of the previous chunk per pair
                if prev_tail is not None and p >= 1:
                    step = next(prev_tail, None)
                    if step is not None:
                        step()
                    else:
                        prev_tail = None
                if p >= PV_LAG:
                    emit_pv(p - PV_LAG)
            for j in range(NP - PV_LAG, NP):
                emit_pv(j)
            if ch == 0:
                prev_tail = tail_steps(ch)
            else:
                final_tail(ch)

    nc.finalize()
    return nc


def get_program() -> bass.Bass:
    global _PROGRAM
    if _PROGRAM is None:
        _PROGRAM = _build_program()
    return _PROGRAM


def make_in_maps(x, Wq, Wk, Wv, gamma):
    """Host-side prep: reshape/slice/rotate, dtype casts, zero-padding, and
    weight-only algebra (G = Wq^T Wk folded; gamma folded into Wv)."""
    x = np.ascontiguousarray(np.asarray(x, dtype=np.float32))
    Wq = np.asarray(Wq, dtype=np.float32)
    Wk = np.asarray(Wk, dtype=np.float32)
    Wv = np.asarray(Wv, dtype=np.float32)
    gamma = float(np.asarray(gamma, dtype=np.float32).reshape(()))

    wg = np.zeros((128, 192), dtype=NP_BF16)
    wg[:C, :C] = (Wk.T @ Wq).astype(NP_BF16)        # gt16: lhsT for kg = G @ xk
    wg[:C, 128 : 128 + C] = (gamma * Wv.T).astype(NP_BF16)   # wv16

    in_maps = []
    for core in range(N_CORES):
        b, h = divmod(core, 2)
        xf = x[b].reshape(C, N)
        xk = xf[:, h * MHALF : (h + 1) * MHALF]
        xo = xf[:, (1 - h) * MHALF : (2 - h) * MHALF]
        # rotate so this core's m-half sits at columns 0:MHALF
        xrot = np.concatenate([xk, xo], axis=1)
        xfp = np.zeros((128, N), dtype=NP_BF16)
        xfp[:C] = xrot.astype(NP_BF16)
        in_maps.append({"xfp": xfp, "wg": wg})
    return in_maps


def gather(results):
    out = np.empty((B, C, N), dtype=np.float32)
    for core in range(N_CORES):
        b, h = divmod(core, 2)
        out[b][:, h * MHALF : (h + 1) * MHALF] = results[core]["out"]
    return out.reshape(B, C, H, W)


def run(inputs, **spmd_kwargs):
    nc = get_program()
    in_maps = make_in_maps(
        inputs["x"], inputs["Wq"], inputs["Wk"], inputs["Wv"], inputs["gamma"]
    )
    res = run_bass_kernel_spmd(nc, in_maps, core_ids=list(range(N_CORES)), **spmd_kwargs)
    return gather(res.results), res


def kernel(x, Wq, Wk, Wv, gamma):
    out, _ = run({"x": x, "Wq": Wq, "Wk": Wk, "Wv": Wv, "gamma": gamma})
    return out
